# revision 7
# baseline (speedup 1.0000x reference)
"""Trainium2 Bass kernel for AtomToEdgeLayer (GNN message passing).

  m = ssp(concat([rbf @ W_rbf.T + b_rbf, vi[idx1], vi[idx0]]) @ W_cat.T + b_cat)

Decomposition (mathematically identical, fp-assoc differences only):
  W_cat = [Wc1 | Wc2 | Wc3] (each [128,128])
  m = ssp(rbf @ (Wc1 @ W_rbf).T + vi[idx1] @ Wc2.T + vi[idx0] @ Wc3.T
          + (b_cat + Wc1 @ b_rbf))
ssp(x) = softplus(x) - ln 2.  The bias rides a ones-row appended to rbf^T.

Sharding: edges split contiguously across 8 cores. Per core, edges are
bucketed by (idx0 < NSPLIT, idx1 < NSPLIT) so gather indices fit int16
(dma_gather requirement), padded per bucket to the tile size T. The device
gathers vi rows (bf16, 256B) from DRAM with the SWDGE dma_gather in transpose
mode, producing feature-major [128k, T] tiles directly usable as matmul lhsT.
Three matmuls accumulate each 128-edge subtile in PSUM; ACT applies softplus
from PSUM; DVE subtracts ln2; DMA stores f32 rows. Host undoes the edge
permutation when assembling the full output.
"""
import os
import sys
import types

sys.path.insert(0, "/opt/trn_rl_repo")

import numpy as np
import ml_dtypes

from concourse import bacc, mybir, tile
from concourse import bass_utils
from concourse.bass_utils import run_bass_kernel_spmd

if "antenv.axon_hooks" not in sys.modules:
    try:
        from trn_agent_boot.trn_boot import _ntff_profile_via_ctypes

        _hook = _ntff_profile_via_ctypes("/opt/axon/libaxon_pjrt.so")
        _mod = types.ModuleType("antenv.axon_hooks")
        _mod.get_axon_ntff_profile_hook = lambda: _hook
        sys.modules["antenv.axon_hooks"] = _mod
    except Exception:
        pass
bass_utils.upload_artifacts = lambda d: d

bf16 = ml_dtypes.bfloat16
LOG2 = float(np.log(2.0))

N_CORES = 8
N, E, D, D_RBF = 50000, 640000, 128, 64
EC = E // N_CORES          # edges per core
NSPLIT = 25000             # atom split so local gather indices fit int16
T = 1024                   # edges per device tile
SUB = T // 128             # 128-edge subtiles per tile
KR = D_RBF + 1             # rbf contraction depth (+1 ones-row carries bias)

LAST_EXEC_NS = None


def _wrap_idx16(idx):
    """[T] -> [128, T//16] int16 wrapped per 16 partitions, replicated x8."""
    w = idx.reshape(-1, 16).T.astype(np.int16)
    return np.tile(w, (8, 1))


def _build(n_tiles, tile_tables, tile_regs):
    """tile_tables[t] = (i_hi, j_hi); tile_regs[t] = valid-index count."""
    nc = bacc.Bacc("TRN2", target_bir_lowering=False, debug=False)
    dt = mybir.dt
    viR_lo = nc.dram_tensor("viR_lo", [NSPLIT, D], dt.bfloat16, kind="ExternalInput").ap()
    viR_hi = nc.dram_tensor("viR_hi", [N - NSPLIT, D], dt.bfloat16, kind="ExternalInput").ap()
    rbfT_d = nc.dram_tensor("rbfT", [n_tiles, KR, T], dt.bfloat16, kind="ExternalInput").ap()
    idx0_d = nc.dram_tensor("idx0", [n_tiles, 128, T // 16], dt.int16, kind="ExternalInput").ap()
    idx1_d = nc.dram_tensor("idx1", [n_tiles, 128, T // 16], dt.int16, kind="ExternalInput").ap()
    wcomb_d = nc.dram_tensor("wcomb", [KR, D], dt.bfloat16, kind="ExternalInput").ap()
    wc2t_d = nc.dram_tensor("wc2t", [D, D], dt.bfloat16, kind="ExternalInput").ap()
    wc3t_d = nc.dram_tensor("wc3t", [D, D], dt.bfloat16, kind="ExternalInput").ap()
    out_d = nc.dram_tensor("out", [n_tiles * T, D], dt.float32, kind="ExternalOutput").ap()

    with tile.TileContext(nc) as tc:
        with (
            tc.tile_pool(name="w", bufs=1) as w_pool,
            tc.tile_pool(name="rbf", bufs=2) as rbf_pool,
            tc.tile_pool(name="idx", bufs=3) as idx_pool,
            tc.tile_pool(name="g", bufs=2) as g_pool,
            tc.tile_pool(name="o", bufs=3) as o_pool,
            tc.tile_pool(name="ps", bufs=2, space="PSUM") as ps_pool,
        ):
            wcomb = w_pool.tile([KR, D], dt.bfloat16, tag="wcomb")
            nc.sync.dma_start(out=wcomb[:], in_=wcomb_d[:])
            wc2t = w_pool.tile([D, D], dt.bfloat16, tag="wc2t")
            nc.sync.dma_start(out=wc2t[:], in_=wc2t_d[:])
            wc3t = w_pool.tile([D, D], dt.bfloat16, tag="wc3t")
            nc.sync.dma_start(out=wc3t[:], in_=wc3t_d[:])
            half = w_pool.tile([128, 1], dt.float32, tag="half")
            nc.gpsimd.memset(half[:], 0.5)

            for t in range(n_tiles):
                i_hi, j_hi = tile_tables[t]
                rbft = rbf_pool.tile([KR, T], dt.bfloat16, tag="rbft")
                nc.sync.dma_start(out=rbft[:], in_=rbfT_d[t])

                it0 = idx_pool.tile([128, T // 16], dt.int16, tag="it0")
                nc.sync.dma_start(out=it0[:], in_=idx0_d[t])
                it1 = idx_pool.tile([128, T // 16], dt.int16, tag="it1")
                nc.sync.dma_start(out=it1[:], in_=idx1_d[t])

                gi = g_pool.tile([128, T], dt.bfloat16, tag="gi")
                nc.gpsimd.dma_gather(
                    gi[:].rearrange("p (one t) -> p one t", one=1),
                    (viR_hi if i_hi else viR_lo)[:],
                    it0[:], num_idxs=T, num_idxs_reg=int(tile_regs[t]), elem_size=D,
                    transpose=True, single_packet=False,
                )
                gj = g_pool.tile([128, T], dt.bfloat16, tag="gj")
                nc.gpsimd.dma_gather(
                    gj[:].rearrange("p (one t) -> p one t", one=1),
                    (viR_hi if j_hi else viR_lo)[:],
                    it1[:], num_idxs=T, num_idxs_reg=int(tile_regs[t]), elem_size=D,
                    transpose=True, single_packet=False,
                )

                for b in range(SUB // 4):  # one PSUM bank = 4 subtiles
                    ps = ps_pool.tile([128, 512], dt.float32, space="PSUM", tag="ps")
                    for s4 in range(4):
                        s = b * 4 + s4
                        sl = slice(s * 128, (s + 1) * 128)
                        col = slice(s4 * 128, (s4 + 1) * 128)
                        nc.tensor.matmul(out=ps[:, col], lhsT=gj[:, sl], rhs=wc2t[:],
                                         start=True, stop=False)
                        nc.tensor.matmul(out=ps[:, col], lhsT=gi[:, sl], rhs=wc3t[:],
                                         start=False, stop=False)
                        nc.tensor.matmul(out=ps[:, col], lhsT=rbft[:, sl], rhs=wcomb[:],
                                         start=False, stop=True)
                    # ssp(x) = softplus(x) - ln2 = ln(0.5*e^x + 0.5)
                    et = o_pool.tile([128, 512], dt.float32, tag="et")
                    nc.scalar.activation(et[:], ps[:],
                                         mybir.ActivationFunctionType.Exp)
                    ot = o_pool.tile([128, 512], dt.float32, tag="ot")
                    nc.scalar.activation(ot[:], et[:],
                                         mybir.ActivationFunctionType.Ln,
                                         scale=0.5, bias=half[:])
                    dst = out_d[t * T + b * 512:t * T + (b + 1) * 512, :]
                    nc.sync.dma_start(
                        out=dst.rearrange("(s p) f -> p s f", p=128),
                        in_=ot[:].rearrange("p (s f) -> p s f", f=128),
                    )
    nc.compile()
    return nc


def kernel(vi, rbf, W_rbf, b_rbf, W_cat, b_cat, edge_index):
    global LAST_EXEC_NS
    vi = np.asarray(vi, dtype=np.float32)
    rbf = np.asarray(rbf, dtype=np.float32)
    W_rbf = np.asarray(W_rbf, dtype=np.float32)
    b_rbf = np.asarray(b_rbf, dtype=np.float32)
    W_cat = np.asarray(W_cat, dtype=np.float32)
    b_cat = np.asarray(b_cat, dtype=np.float32)
    edge_index = np.asarray(edge_index)

    # ---- weight folding ----
    Wc1, Wc2, Wc3 = W_cat[:, :D], W_cat[:, D:2 * D], W_cat[:, 2 * D:]
    W_comb = Wc1 @ W_rbf                                   # [D, D_RBF]
    b_tot = b_cat + Wc1 @ b_rbf                            # [D]
    wcomb_rhs = np.concatenate([W_comb.T, b_tot[None, :]], axis=0).astype(bf16)
    wc2t = np.ascontiguousarray(Wc2.T).astype(bf16)
    wc3t = np.ascontiguousarray(Wc3.T).astype(bf16)

    viR = vi.astype(bf16)
    viR_lo = np.ascontiguousarray(viR[:NSPLIT])
    viR_hi = np.ascontiguousarray(viR[NSPLIT:])

    idx0 = edge_index[0].astype(np.int64)
    idx1 = edge_index[1].astype(np.int64)

    # ---- per-core bucketing ----
    core_sel = []          # core -> bucket -> ordered edge positions (core-rel)
    for c in range(N_CORES):
        lo, hi = c * EC, (c + 1) * EC
        bucket = (idx0[lo:hi] >= NSPLIT).astype(np.int8) * 2 + \
                 (idx1[lo:hi] >= NSPLIT).astype(np.int8)
        core_sel.append([np.nonzero(bucket == bk)[0] for bk in range(4)])

    per_bucket_tiles = [
        max((core_sel[c][bk].size + T - 1) // T for c in range(N_CORES))
        for bk in range(4)
    ]
    tile_tables = []
    for bk in range(4):
        tile_tables += [(bk >= 2, bk % 2 == 1)] * per_bucket_tiles[bk]
    n_tiles = len(tile_tables)

    # per-tile valid-count = max over cores (graph, incl. num_idxs_reg, is shared)
    tile_regs = []
    for bk in range(4):
        want = per_bucket_tiles[bk]
        for s in range(want):
            v = max(min(max(core_sel[c][bk].size - s * T, 0), T) for c in range(N_CORES))
            tile_regs.append(max(v, 1))

    in_maps, perms = [], []
    for c in range(N_CORES):
        lo = c * EC
        i0, i1 = idx0[lo:lo + EC], idx1[lo:lo + EC]
        rbf_c = rbf[lo:lo + EC]
        rbf_tiles, i0_tiles, i1_tiles, rows = [], [], [], []
        ti = 0
        for bk in range(4):
            sel = core_sel[c][bk]
            want = per_bucket_tiles[bk]
            sel_pad = np.concatenate([sel, np.full(want * T - sel.size, -1, np.int64)])
            for s in range(0, want * T, T):
                chunk = sel_pad[s:s + T]
                valid = chunk >= 0
                reg = tile_regs[ti]; ti += 1
                safe = np.where(valid, chunk, 0)
                li = np.where(valid, i0[safe] - (NSPLIT if bk >= 2 else 0), 0)
                lj = np.where(valid, i1[safe] - (NSPLIT if bk % 2 == 1 else 0), 0)
                # beyond the shared valid count: -1 stops Q7 desc-gen early
                tail = np.arange(T) >= reg
                li[tail] = -1
                lj[tail] = -1
                rb = np.zeros((T, D_RBF), np.float32)
                rb[valid] = rbf_c[chunk[valid]]
                rt = np.concatenate([rb.T, np.ones((1, T), np.float32)], axis=0)
                rbf_tiles.append(rt.astype(bf16))
                i0_tiles.append(_wrap_idx16(li))
                i1_tiles.append(_wrap_idx16(lj))
                rows.append(chunk)
        in_maps.append({
            "viR_lo": viR_lo, "viR_hi": viR_hi,
            "rbfT": np.stack(rbf_tiles),
            "idx0": np.stack(i0_tiles),
            "idx1": np.stack(i1_tiles),
            "wcomb": wcomb_rhs, "wc2t": wc2t, "wc3t": wc3t,
        })
        perms.append(np.concatenate(rows))

    nc = _build(n_tiles, tile_tables, tile_regs)
    if os.environ.get("BENCH"):
        res = run_bass_kernel_spmd(nc, in_maps, core_ids=list(range(N_CORES)),
                                   trace=True, trace_cores=[0])
        LAST_EXEC_NS = res.exec_time_ns
    else:
        res = run_bass_kernel_spmd(nc, in_maps, core_ids=list(range(N_CORES)))

    out = np.empty((E, D), np.float32)
    for c in range(N_CORES):
        dev = res.results[c]["out"]
        perm = perms[c]
        valid = perm >= 0
        out[c * EC + perm[valid]] = dev[valid]
    return out


# revision 8
# speedup vs baseline: 1.2392x; 1.2392x over previous
"""Trainium2 Bass kernel for AtomToEdgeLayer (GNN message passing).

  m = ssp(concat([rbf @ W_rbf.T + b_rbf, vi[idx1], vi[idx0]]) @ W_cat.T + b_cat)

Decomposition (mathematically identical, fp-assoc differences only):
  W_cat = [Wc1 | Wc2 | Wc3] (each [128,128])
  m = ssp(rbf @ (Wc1 @ W_rbf).T + vi[idx1] @ Wc2.T + vi[idx0] @ Wc3.T
          + (b_cat + Wc1 @ b_rbf))
ssp(x) = softplus(x) - ln 2.  The bias rides a ones-row appended to rbf^T.

Sharding: edges split contiguously across 8 cores. Per core, edges are
bucketed by (idx0 < NSPLIT, idx1 < NSPLIT) so gather indices fit int16
(dma_gather requirement), padded per bucket to the tile size T. The device
gathers vi rows (bf16, 256B) from DRAM with the SWDGE dma_gather in transpose
mode, producing feature-major [128k, T] tiles directly usable as matmul lhsT.
Three matmuls accumulate each 128-edge subtile in PSUM; ACT applies softplus
from PSUM; DVE subtracts ln2; DMA stores f32 rows. Host undoes the edge
permutation when assembling the full output.
"""
import os
import sys
import types

sys.path.insert(0, "/opt/trn_rl_repo")

import numpy as np
import ml_dtypes

from concourse import bacc, mybir, tile
from concourse import bass_utils
from concourse.bass_utils import run_bass_kernel_spmd

if "antenv.axon_hooks" not in sys.modules:
    try:
        from trn_agent_boot.trn_boot import _ntff_profile_via_ctypes

        _hook = _ntff_profile_via_ctypes("/opt/axon/libaxon_pjrt.so")
        _mod = types.ModuleType("antenv.axon_hooks")
        _mod.get_axon_ntff_profile_hook = lambda: _hook
        sys.modules["antenv.axon_hooks"] = _mod
    except Exception:
        pass
bass_utils.upload_artifacts = lambda d: d

bf16 = ml_dtypes.bfloat16
LOG2 = float(np.log(2.0))

N_CORES = 8
N, E, D, D_RBF = 50000, 640000, 128, 64
EC = E // N_CORES          # edges per core
NSPLIT = 25000             # atom split so local gather indices fit int16
T = 2048                   # edges per device tile
SUB = T // 128             # 128-edge subtiles per tile
KR = D_RBF + 1             # rbf contraction depth (+1 ones-row carries bias)

LAST_EXEC_NS = None


def _wrap_idx16(idx):
    """[T] -> [128, T//16] int16 wrapped per 16 partitions, replicated x8."""
    w = idx.reshape(-1, 16).T.astype(np.int16)
    return np.tile(w, (8, 1))


def _build(n_tiles, tile_tables, tile_regs):
    """tile_tables[t] = (i_hi, j_hi); tile_regs[t] = valid-index count."""
    nc = bacc.Bacc("TRN2", target_bir_lowering=False, debug=False)
    dt = mybir.dt
    viR_lo = nc.dram_tensor("viR_lo", [NSPLIT, D], dt.bfloat16, kind="ExternalInput").ap()
    viR_hi = nc.dram_tensor("viR_hi", [N - NSPLIT, D], dt.bfloat16, kind="ExternalInput").ap()
    rbfT_d = nc.dram_tensor("rbfT", [n_tiles, KR, T], dt.bfloat16, kind="ExternalInput").ap()
    idx0_d = nc.dram_tensor("idx0", [n_tiles, 128, T // 16], dt.int16, kind="ExternalInput").ap()
    idx1_d = nc.dram_tensor("idx1", [n_tiles, 128, T // 16], dt.int16, kind="ExternalInput").ap()
    wcomb_d = nc.dram_tensor("wcomb", [KR, D], dt.bfloat16, kind="ExternalInput").ap()
    wc2t_d = nc.dram_tensor("wc2t", [D, D], dt.bfloat16, kind="ExternalInput").ap()
    wc3t_d = nc.dram_tensor("wc3t", [D, D], dt.bfloat16, kind="ExternalInput").ap()
    out_d = nc.dram_tensor("out", [n_tiles * T, D], dt.float32, kind="ExternalOutput").ap()

    with tile.TileContext(nc) as tc:
        with (
            tc.tile_pool(name="w", bufs=1) as w_pool,
            tc.tile_pool(name="rbf", bufs=2) as rbf_pool,
            tc.tile_pool(name="idx", bufs=3) as idx_pool,
            tc.tile_pool(name="g", bufs=2) as g_pool,
            tc.tile_pool(name="o", bufs=3) as o_pool,
            tc.tile_pool(name="ps", bufs=2, space="PSUM") as ps_pool,
        ):
            wcomb = w_pool.tile([KR, D], dt.bfloat16, tag="wcomb")
            nc.sync.dma_start(out=wcomb[:], in_=wcomb_d[:])
            wc2t = w_pool.tile([D, D], dt.bfloat16, tag="wc2t")
            nc.sync.dma_start(out=wc2t[:], in_=wc2t_d[:])
            wc3t = w_pool.tile([D, D], dt.bfloat16, tag="wc3t")
            nc.sync.dma_start(out=wc3t[:], in_=wc3t_d[:])
            half = w_pool.tile([128, 1], dt.float32, tag="half")
            nc.gpsimd.memset(half[:], 0.5)

            for t in range(n_tiles):
                i_hi, j_hi = tile_tables[t]
                rbft = rbf_pool.tile([KR, T], dt.bfloat16, tag="rbft")
                nc.sync.dma_start(out=rbft[:], in_=rbfT_d[t])

                it0 = idx_pool.tile([128, T // 16], dt.int16, tag="it0")
                nc.sync.dma_start(out=it0[:], in_=idx0_d[t])
                it1 = idx_pool.tile([128, T // 16], dt.int16, tag="it1")
                nc.sync.dma_start(out=it1[:], in_=idx1_d[t])

                gi = g_pool.tile([128, T], dt.bfloat16, tag="gi")
                nc.gpsimd.dma_gather(
                    gi[:].rearrange("p (one t) -> p one t", one=1),
                    (viR_hi if i_hi else viR_lo)[:],
                    it0[:], num_idxs=T, num_idxs_reg=int(tile_regs[t]), elem_size=D,
                    transpose=True, single_packet=False,
                )
                gj = g_pool.tile([128, T], dt.bfloat16, tag="gj")
                nc.gpsimd.dma_gather(
                    gj[:].rearrange("p (one t) -> p one t", one=1),
                    (viR_hi if j_hi else viR_lo)[:],
                    it1[:], num_idxs=T, num_idxs_reg=int(tile_regs[t]), elem_size=D,
                    transpose=True, single_packet=False,
                )

                for b in range(SUB // 4):  # one PSUM bank = 4 subtiles
                    ps = ps_pool.tile([128, 512], dt.float32, space="PSUM", tag="ps")
                    for s4 in range(4):
                        s = b * 4 + s4
                        sl = slice(s * 128, (s + 1) * 128)
                        col = slice(s4 * 128, (s4 + 1) * 128)
                        nc.tensor.matmul(out=ps[:, col], lhsT=gj[:, sl], rhs=wc2t[:],
                                         start=True, stop=False)
                        nc.tensor.matmul(out=ps[:, col], lhsT=gi[:, sl], rhs=wc3t[:],
                                         start=False, stop=False)
                        nc.tensor.matmul(out=ps[:, col], lhsT=rbft[:, sl], rhs=wcomb[:],
                                         start=False, stop=True)
                    # ssp(x) = softplus(x) - ln2 = ln(0.5*e^x + 0.5)
                    et = o_pool.tile([128, 512], dt.float32, tag="et")
                    nc.scalar.activation(et[:], ps[:],
                                         mybir.ActivationFunctionType.Exp)
                    ot = o_pool.tile([128, 512], dt.float32, tag="ot")
                    nc.scalar.activation(ot[:], et[:],
                                         mybir.ActivationFunctionType.Ln,
                                         scale=0.5, bias=half[:])
                    dst = out_d[t * T + b * 512:t * T + (b + 1) * 512, :]
                    nc.sync.dma_start(
                        out=dst.rearrange("(s p) f -> p s f", p=128),
                        in_=ot[:].rearrange("p (s f) -> p s f", f=128),
                    )
    nc.compile()
    return nc


def kernel(vi, rbf, W_rbf, b_rbf, W_cat, b_cat, edge_index):
    global LAST_EXEC_NS
    vi = np.asarray(vi, dtype=np.float32)
    rbf = np.asarray(rbf, dtype=np.float32)
    W_rbf = np.asarray(W_rbf, dtype=np.float32)
    b_rbf = np.asarray(b_rbf, dtype=np.float32)
    W_cat = np.asarray(W_cat, dtype=np.float32)
    b_cat = np.asarray(b_cat, dtype=np.float32)
    edge_index = np.asarray(edge_index)

    # ---- weight folding ----
    Wc1, Wc2, Wc3 = W_cat[:, :D], W_cat[:, D:2 * D], W_cat[:, 2 * D:]
    W_comb = Wc1 @ W_rbf                                   # [D, D_RBF]
    b_tot = b_cat + Wc1 @ b_rbf                            # [D]
    wcomb_rhs = np.concatenate([W_comb.T, b_tot[None, :]], axis=0).astype(bf16)
    wc2t = np.ascontiguousarray(Wc2.T).astype(bf16)
    wc3t = np.ascontiguousarray(Wc3.T).astype(bf16)

    viR = vi.astype(bf16)
    viR_lo = np.ascontiguousarray(viR[:NSPLIT])
    viR_hi = np.ascontiguousarray(viR[NSPLIT:])

    idx0 = edge_index[0].astype(np.int64)
    idx1 = edge_index[1].astype(np.int64)

    # ---- per-core bucketing ----
    core_sel = []          # core -> bucket -> ordered edge positions (core-rel)
    for c in range(N_CORES):
        lo, hi = c * EC, (c + 1) * EC
        bucket = (idx0[lo:hi] >= NSPLIT).astype(np.int8) * 2 + \
                 (idx1[lo:hi] >= NSPLIT).astype(np.int8)
        core_sel.append([np.nonzero(bucket == bk)[0] for bk in range(4)])

    per_bucket_tiles = [
        max((core_sel[c][bk].size + T - 1) // T for c in range(N_CORES))
        for bk in range(4)
    ]
    tile_tables = []
    for bk in range(4):
        tile_tables += [(bk >= 2, bk % 2 == 1)] * per_bucket_tiles[bk]
    n_tiles = len(tile_tables)

    # per-tile valid-count = max over cores (graph, incl. num_idxs_reg, is shared)
    tile_regs = []
    for bk in range(4):
        want = per_bucket_tiles[bk]
        for s in range(want):
            v = max(min(max(core_sel[c][bk].size - s * T, 0), T) for c in range(N_CORES))
            tile_regs.append(max(v, 1))

    in_maps, perms = [], []
    for c in range(N_CORES):
        lo = c * EC
        i0, i1 = idx0[lo:lo + EC], idx1[lo:lo + EC]
        rbf_c = rbf[lo:lo + EC]
        rbf_tiles, i0_tiles, i1_tiles, rows = [], [], [], []
        ti = 0
        for bk in range(4):
            sel = core_sel[c][bk]
            want = per_bucket_tiles[bk]
            sel_pad = np.concatenate([sel, np.full(want * T - sel.size, -1, np.int64)])
            for s in range(0, want * T, T):
                chunk = sel_pad[s:s + T]
                valid = chunk >= 0
                reg = tile_regs[ti]; ti += 1
                safe = np.where(valid, chunk, 0)
                li = np.where(valid, i0[safe] - (NSPLIT if bk >= 2 else 0), 0)
                lj = np.where(valid, i1[safe] - (NSPLIT if bk % 2 == 1 else 0), 0)
                # beyond the shared valid count: -1 stops Q7 desc-gen early
                tail = np.arange(T) >= reg
                li[tail] = -1
                lj[tail] = -1
                rb = np.zeros((T, D_RBF), np.float32)
                rb[valid] = rbf_c[chunk[valid]]
                rt = np.concatenate([rb.T, np.ones((1, T), np.float32)], axis=0)
                rbf_tiles.append(rt.astype(bf16))
                i0_tiles.append(_wrap_idx16(li))
                i1_tiles.append(_wrap_idx16(lj))
                rows.append(chunk)
        in_maps.append({
            "viR_lo": viR_lo, "viR_hi": viR_hi,
            "rbfT": np.stack(rbf_tiles),
            "idx0": np.stack(i0_tiles),
            "idx1": np.stack(i1_tiles),
            "wcomb": wcomb_rhs, "wc2t": wc2t, "wc3t": wc3t,
        })
        perms.append(np.concatenate(rows))

    nc = _build(n_tiles, tile_tables, tile_regs)
    if os.environ.get("BENCH"):
        res = run_bass_kernel_spmd(nc, in_maps, core_ids=list(range(N_CORES)),
                                   trace=True, trace_cores=[0])
        LAST_EXEC_NS = res.exec_time_ns
    else:
        res = run_bass_kernel_spmd(nc, in_maps, core_ids=list(range(N_CORES)))

    out = np.empty((E, D), np.float32)
    for c in range(N_CORES):
        dev = res.results[c]["out"]
        perm = perms[c]
        valid = perm >= 0
        out[c * EC + perm[valid]] = dev[valid]
    return out
